# revision 6
# baseline (speedup 1.0000x reference)
"""AdditiveAttention (Bahdanau) distributed Bass kernel for 8 TRN2 NeuronCores.

Reference computation (per batch b):
    qp = queries[b] @ W_q                  # [Q, H]
    kp = keys[b]    @ W_k                  # [K, H]
    S[q,k]  = sum_h w_v[h] * tanh(qp[q,h] + kp[k,h])
    S masked to -1e6 for k >= valid_lens[b]
    attn = softmax(S, axis=k)
    out[b] = attn @ values[b]              # [Q, DV]

Key idea: tanh is replaced by a short sine series fit under the Gaussian
weight of x = qp+kp ~ N(0, 2):

    tanh(x) ~= sum_r a_r sin(w_r x)

Each sine term FACTORIZES across q and k:

    sin(w_r (qp+kp)) = sin(w_r qp) cos(w_r kp) + cos(w_r qp) sin(w_r kp)

so the [Q,K,H] pointwise tanh pass (the ACT-engine bottleneck of the
direct approach) collapses into 2R rank-H matmuls on the PE array:

    S^T[k,q] = sum_r sum_h [ sin_r(kp)[h,k] * (a_r w_v cos_r(qp))[h,q]
                           + cos_r(kp)[h,k] * (a_r w_v sin_r(qp))[h,q] ]

The ACT Sin spline is only accurate on |arg| <= ~4.2, and |proj| reaches
~5, so only the base frequencies {w0, 2w0, 3w0} are evaluated directly
(args <= ~5.1; out-of-domain hits are ~1-per-16k-tile and tiny).  The
higher frequencies {4w0, 6w0, 8w0} are derived on DVE with exact
double-angle identities (no large-argument sin ever evaluated):

    sin 2a = 2 sin a cos a,   cos 2a = 1 - 2 sin^2 a

Sin lives in the trig_and_small ACT table set, Exp in exp_and_others:
ALL sin calls are emitted before ALL exp calls so the ~2.7us table
switch happens once.

Per core: 2 full batches (16/8), each 1 q-unit + 4 k-blocks of 128.
Per unit: PE projects (stationary = weights), PE makes scaled copies
X[h,(r,pos)] = w_r proj[pos,h] via a constant selector matmul, 2 ACT
calls give sin(X) and cos(X) (bias=pi/2), DVE derives the high blocks.
q-side features are scaled by a_r*w_v[h] on DVE (constant pattern).
S^T accumulates in PSUM via 12 chained matmuls per k-block (emitted
derived-blocks-first so each PE instruction picks up at most one new
semaphore), is parked in SBUF (f32), then masked Exp and
[numer | denom] = E^T.T @ [V | 1].  Host divides.

exp needs no max-subtraction: |S| <= sum_r |a_r| * ||w_v||_1 ~ 15, and
masked positions get bias -30000 -> exp == 0 exactly.
"""

import math
import os

import numpy as np

import concourse.bacc as bacc
import concourse.bass as bass
import concourse.tile as tile
from concourse import mybir
from concourse.bass_utils import run_bass_kernel_spmd

B, Q, K, QS, KS, H, DV = 16, 128, 512, 256, 256, 128, 256
N_CORES = 8
NB = B // N_CORES  # batches per core
NT = K // 128      # k blocks per batch
MASK_NEG = -30000.0

# Base frequencies (bf16-exact) evaluated by ACT Sin; effective basis is
# {w1, w2, w3, 2*w2, 2*w3, 4*w2} after DVE double-angle derivation.
# COEF fit against that exact basis under N(0, 1.5^2) weight on [-10,10].
WBASE = [0.33984375, 0.6796875, 1.0234375]
COEF = [1.1344966112424597, 0.1414215634345795, 0.12195299983389567,
        0.142637682916146, 0.05109580923482298, 0.025410122618652033]
NF = 6           # total frequency blocks
NBASE = 3        # ACT-evaluated base blocks
FW = NF * 128    # feature width (768)
XW = NBASE * 128  # selector/X width (384)

F32 = mybir.dt.float32
BF16 = mybir.dt.bfloat16
MULT = mybir.AluOpType.mult
ADD = mybir.AluOpType.add

_BUILD_CACHE: dict = {}
LAST_RESULT = None  # BassKernelResults of the most recent run (for timing)


def _derive_high_blocks(nc, scr_pool, s, c, tag):
    """Fill feature blocks 3..5 (freqs 4,6,8) of sin tile `s` / cos tile
    `c` from ACT-computed blocks 0..2 (freqs 1,2,3) via double angles.
    Tiles are [128, U, 768] (U units); ops run across all units at once.
    """
    u = s.shape[1]
    b1 = (slice(None), slice(None), slice(128, 256))    # freq 2
    b4 = (slice(None), slice(None), slice(384, 512))    # freq 4 (out)
    b34 = (slice(None), slice(None), slice(256, 512))   # freqs 3,4
    b68 = (slice(None), slice(None), slice(512, 768))   # freqs 6,8 (out)
    # freq 4 = double of freq 2
    w2 = scr_pool.tile([128, u, 128], BF16, tag=f"{tag}w2")
    nc.vector.tensor_tensor(w2, s[b1], s[b1], op=MULT)
    nc.vector.tensor_scalar(c[b4], w2, -2.0, 1.0, op0=MULT, op1=ADD)
    nc.vector.scalar_tensor_tensor(s[b4], s[b1], 2.0, c[b1],
                                   op0=MULT, op1=MULT)
    # freqs 6,8 = doubles of freqs 3,4 (contiguous pair)
    w34 = scr_pool.tile([128, u, 256], BF16, tag=f"{tag}w34")
    nc.vector.tensor_tensor(w34, s[b34], s[b34], op=MULT)
    nc.vector.tensor_scalar(c[b68], w34, -2.0, 1.0, op0=MULT, op1=ADD)
    nc.vector.scalar_tensor_tensor(s[b68], s[b34], 2.0, c[b34],
                                   op0=MULT, op1=MULT)


def _build() -> bass.Bass:
    nc = bacc.Bacc()

    qT = nc.declare_dram_parameter("qT", [NB, QS, Q], BF16, isOutput=False)
    kT = nc.declare_dram_parameter("kT", [NB, KS, K], BF16, isOutput=False)
    vv = nc.declare_dram_parameter("vv", [NB, K, DV], BF16, isOutput=False)
    mb = nc.declare_dram_parameter("mb", [128, NB * NT], F32, isOutput=False)
    wq = nc.declare_dram_parameter("wq", [QS, H], BF16, isOutput=False)
    wk = nc.declare_dram_parameter("wk", [KS, H], BF16, isOutput=False)
    sel = nc.declare_dram_parameter("sel", [128, XW], BF16, isOutput=False)
    wva = nc.declare_dram_parameter("wva", [128, FW], BF16, isOutput=False)
    onum = nc.declare_dram_parameter("onum", [NB, Q, DV], F32, isOutput=True)
    oden = nc.declare_dram_parameter("oden", [NB, Q], F32, isOutput=True)

    ND = QS // 128  # 128-row blocks in the projection contraction dim

    with tile.TileContext(nc) as tc:
        with (
            tc.tile_pool(name="consts", bufs=1) as consts,
            tc.tile_pool(name="stg", bufs=1) as stg,
            tc.tile_pool(name="io", bufs=1) as io,
            tc.tile_pool(name="feat", bufs=1) as feat,
            tc.tile_pool(name="scr", bufs=2) as scr,
            tc.tile_pool(name="mid", bufs=2) as mid,
            tc.tile_pool(name="px", bufs=2, space="PSUM") as px,
            tc.tile_pool(name="pproj", bufs=2, space="PSUM") as pproj,
            tc.tile_pool(name="pst", bufs=2, space="PSUM") as pst,
            tc.tile_pool(name="pout", bufs=2, space="PSUM") as pout,
        ):
            # ---- one-time constants ----
            # PE inputs are staged through a DVE copy so PE instructions
            # wait only on the DVE (or ACT) semaphore.
            sel_s = consts.tile([128, XW], BF16)
            nc.sync.dma_start(out=sel_s, in_=sel[:])
            sel_b = consts.tile([128, XW], BF16)
            nc.vector.tensor_copy(sel_b, sel_s)

            wva_b = consts.tile([128, FW], BF16)  # read by DVE only
            nc.sync.dma_start(out=wva_b, in_=wva[:])

            wq_s = consts.tile([128, ND, H], BF16)
            nc.sync.dma_start(out=wq_s, in_=wq.rearrange("(n p) h -> p n h", p=128))
            wq_b = consts.tile([128, ND, H], BF16)
            nc.vector.tensor_copy(wq_b, wq_s)
            wk_s = consts.tile([128, ND, H], BF16)
            nc.sync.dma_start(out=wk_s, in_=wk.rearrange("(n p) h -> p n h", p=128))
            wk_b = consts.tile([128, ND, H], BF16)
            nc.vector.tensor_copy(wk_b, wk_s)

            mb_b = consts.tile([128, NB * NT], F32)  # read by ACT (bias)
            nc.sync.dma_start(out=mb_b, in_=mb[:])

            pih = consts.tile([128, 1], F32)  # pi/2 bias column for cos
            nc.vector.memset(pih, math.pi / 2)

            # ---- featurize both batches (ACT: Sin only) ----
            qsf = [None] * NB
            qcf = [None] * NB
            kS = [None] * NB
            kC = [None] * NB
            v_b = [[None] * NT for _ in range(NB)]

            for j in range(NB):
                qT_in = qT[j].rearrange("(n p) q -> p n q", p=128)
                kT_in = kT[j].rearrange("(n p) k -> p n k", p=128)
                v_in = vv[j].rearrange("(t p) d -> p t d", p=128)

                qT_b = []
                for n in range(ND):
                    qs = stg.tile([128, Q], BF16, tag=f"qs{j}{n}")
                    nc.sync.dma_start(out=qs, in_=qT_in[:, n, :])
                    qb = io.tile([128, Q], BF16, tag=f"qb{j}{n}")
                    nc.vector.tensor_copy(qb, qs)
                    qT_b.append(qb)
                kT_b = []
                for n in range(ND):
                    ks = stg.tile([128, K], BF16, tag=f"ks{j}{n}")
                    nc.sync.dma_start(out=ks[:, : K // 2], in_=kT_in[:, n, : K // 2])
                    nc.sync.dma_start(out=ks[:, K // 2 :], in_=kT_in[:, n, K // 2 :])
                    kb = io.tile([128, K], BF16, tag=f"kb{j}{n}")
                    nc.vector.tensor_copy(kb, ks)
                    kT_b.append(kb)
                for t in range(NT):
                    vs = stg.tile([128, DV], BF16, tag=f"vs{j}{t}")
                    nc.sync.dma_start(out=vs, in_=v_in[:, t, :])
                    vb = io.tile([128, DV + 1], BF16, tag=f"vb{j}{t}")
                    nc.vector.tensor_copy(vb[:, :DV], vs)
                    nc.vector.memset(vb[:, DV : DV + 1], 1.0)
                    v_b[j][t] = vb

                # ---- q unit ----
                qp_ps = pproj.tile([Q, H], F32, tag="proj")
                for n in range(ND):
                    nc.tensor.matmul(
                        qp_ps, lhsT=qT_b[n], rhs=wq_b[:, n, :],
                        start=(n == 0), stop=(n == ND - 1),
                    )
                qp_sb = mid.tile([Q, H], BF16, tag="proj_sb")
                nc.vector.tensor_copy(qp_sb, qp_ps)

                xq_ps = px.tile([H, XW], F32, tag="x")
                nc.tensor.matmul(xq_ps, lhsT=qp_sb, rhs=sel_b, start=True, stop=True)
                qsin = feat.tile([H, 1, FW], BF16, tag=f"qsin{j}")
                nc.scalar.activation(
                    out=qsin[:, 0, :XW], in_=xq_ps,
                    func=mybir.ActivationFunctionType.Sin,
                )
                qcos = feat.tile([H, 1, FW], BF16, tag=f"qcos{j}")
                nc.scalar.activation(
                    out=qcos[:, 0, :XW], in_=xq_ps,
                    func=mybir.ActivationFunctionType.Sin, bias=pih,
                )
                _derive_high_blocks(nc, scr, qsin, qcos, f"q{j}")
                # fold a_r * w_v[h] into the q features (DVE)
                qsf_t = feat.tile([H, FW], BF16, tag=f"qsf{j}")
                nc.vector.tensor_tensor(qsf_t, qsin[:, 0, :], wva_b, op=MULT)
                qcf_t = feat.tile([H, FW], BF16, tag=f"qcf{j}")
                nc.vector.tensor_tensor(qcf_t, qcos[:, 0, :], wva_b, op=MULT)
                qsf[j], qcf[j] = qsf_t, qcf_t

                # ---- k units ----
                ksin = feat.tile([H, NT, FW], BF16, tag=f"ksin{j}")
                kcos = feat.tile([H, NT, FW], BF16, tag=f"kcos{j}")
                for t in range(NT):
                    kp_ps = pproj.tile([128, H], F32, tag="proj")
                    for n in range(ND):
                        nc.tensor.matmul(
                            kp_ps, lhsT=kT_b[n][:, t * 128 : (t + 1) * 128],
                            rhs=wk_b[:, n, :],
                            start=(n == 0), stop=(n == ND - 1),
                        )
                    kp_sb = mid.tile([128, H], BF16, tag="proj_sb")
                    nc.vector.tensor_copy(kp_sb, kp_ps)

                    xk_ps = px.tile([H, XW], F32, tag="x")
                    nc.tensor.matmul(
                        xk_ps, lhsT=kp_sb, rhs=sel_b, start=True, stop=True,
                    )
                    nc.scalar.activation(
                        out=ksin[:, t, :XW], in_=xk_ps,
                        func=mybir.ActivationFunctionType.Sin,
                    )
                    nc.scalar.activation(
                        out=kcos[:, t, :XW], in_=xk_ps,
                        func=mybir.ActivationFunctionType.Sin, bias=pih,
                    )
                _derive_high_blocks(nc, scr, ksin, kcos, f"k{j}")
                kS[j], kC[j] = ksin, kcos

            # ---- S^T chains (PE), parked in SBUF f32 ----
            st_sb = [[None] * NT for _ in range(NB)]
            # derived blocks (DVE-produced) first: the first matmul of a
            # chain then has both operands on the DVE semaphore.
            border = [3, 4, 5, 0, 1, 2]
            for j in range(NB):
                for t in range(NT):
                    st_ps = pst.tile([128, Q], F32, tag="st")
                    for bi, rb in enumerate(border):
                        rsl = slice(rb * 128, (rb + 1) * 128)
                        nc.tensor.matmul(
                            st_ps, lhsT=kS[j][:, t, rsl], rhs=qcf[j][:, rsl],
                            start=(bi == 0), stop=False,
                        )
                        nc.tensor.matmul(
                            st_ps, lhsT=kC[j][:, t, rsl], rhs=qsf[j][:, rsl],
                            start=False, stop=(bi == len(border) - 1),
                        )
                    ss = feat.tile([128, Q], F32, tag=f"st{j}{t}")
                    nc.vector.tensor_copy(ss, st_ps)
                    st_sb[j][t] = ss

            # ---- Exp + output (ACT: Exp only) ----
            for j in range(NB):
                e_sb = [None] * NT
                for t in range(NT):
                    eb = mid.tile([128, Q], BF16, tag=f"e{t}")
                    nc.scalar.activation(
                        out=eb, in_=st_sb[j][t],
                        func=mybir.ActivationFunctionType.Exp,
                        bias=mb_b[:, j * NT + t : j * NT + t + 1],
                    )
                    e_sb[t] = eb

                o_ps = pout.tile([Q, DV + 1], F32, tag="o")
                for t in range(NT):
                    nc.tensor.matmul(
                        o_ps, lhsT=e_sb[t], rhs=v_b[j][t],
                        start=(t == 0), stop=(t == NT - 1),
                    )
                o_sb = mid.tile([Q, DV + 1], F32, tag="osb")
                nc.vector.tensor_copy(o_sb, o_ps)
                hd = DV // 2
                nc.sync.dma_start(out=onum[j][:, :hd], in_=o_sb[:, :hd])
                nc.sync.dma_start(out=onum[j][:, hd:DV], in_=o_sb[:, hd:DV])
                nc.sync.dma_start(out=oden[j], in_=o_sb[:, DV : DV + 1])

    nc.finalize()
    return nc


def kernel(queries, keys, values, valid_lens, W_q, W_k, w_v):
    import ml_dtypes

    queries = np.asarray(queries, dtype=np.float32)
    keys = np.asarray(keys, dtype=np.float32)
    values = np.asarray(values, dtype=np.float32)
    W_q = np.asarray(W_q, dtype=np.float32)
    W_k = np.asarray(W_k, dtype=np.float32)
    w_v = np.asarray(w_v, dtype=np.float32)
    vl = np.asarray(valid_lens).astype(np.int64)

    nc = _BUILD_CACHE.get("v3")
    if nc is None:
        nc = _build()
        _BUILD_CACHE["v3"] = nc

    bf = ml_dtypes.bfloat16
    sel_np = np.zeros((128, XW), bf)
    for r in range(NBASE):
        sel_np[np.arange(128), r * 128 + np.arange(128)] = np.float32(WBASE[r])
    wva_np = np.zeros((128, FW), bf)
    for r in range(NF):
        wva_np[:, r * 128 : (r + 1) * 128] = np.float32(COEF[r]) * w_v[:, None]

    kidx = np.arange(128)
    in_maps = []
    for c in range(N_CORES):
        qTp = np.zeros((NB, QS, Q), bf)
        kTp = np.zeros((NB, KS, K), bf)
        vpp = np.zeros((NB, K, DV), bf)
        mbp = np.zeros((128, NB * NT), np.float32)
        for j in range(NB):
            b = c * NB + j
            qTp[j] = queries[b].T
            kTp[j] = keys[b].T
            vpp[j] = values[b]
            for t in range(NT):
                mbp[:, j * NT + t] = np.where(
                    t * 128 + kidx < vl[b], 0.0, MASK_NEG
                )
        in_maps.append(
            {
                "qT": qTp,
                "kT": kTp,
                "vv": vpp,
                "mb": mbp,
                "wq": W_q.astype(bf),
                "wk": W_k.astype(bf),
                "sel": sel_np,
                "wva": wva_np,
            }
        )

    global LAST_RESULT
    res = run_bass_kernel_spmd(
        nc,
        in_maps,
        core_ids=list(range(N_CORES)),
        trace=bool(os.environ.get("KERNEL_TRACE")),
    )
    LAST_RESULT = res

    out = np.zeros((B, Q, DV), np.float32)
    for c in range(N_CORES):
        onum = res.results[c]["onum"].astype(np.float64)
        oden = res.results[c]["oden"].astype(np.float64)
        for j in range(NB):
            out[c * NB + j] = onum[j] / oden[j][:, None]
    return out.astype(np.float32)
